# revision 1
# baseline (speedup 1.0000x reference)
"""Trainium2 Bass kernel for a transformer block (dense_transformer).

Reference computation (B=2, N=2048, C=1024, 16 heads, head_dim=64):
    x = x + attn(LN1(x))            # attn WITHOUT output projection; heads
                                    # interleaved by the faithful reshape
    out = x + MLP(LN2(x))           # MLP = relu(x@w1+b1)@w2+b2

Sharding: 8 cores; core c handles batch b=c//4 and heads 4g..4g+3 (g=c%4).
Because the reference reshapes [B,H,N,hd]->[B,N,C] without transposing
heads back, head h's attention output occupies output rows
[128h, 128h+128) of batch b: out[b, 128h+r, 64m+d] = attn_h[16r+m, d].
So a (batch, 4-head) shard produces a contiguous [512, 1024] output slab
and the whole residual+MLP for those rows is local to the core.

Per-core pipeline (all matmuls on PE, fp32 accumulate in PSUM; fp32r
operands stream at full PE rate for N>=256 and carry ~1e-4 relative
error vs bf16's ~2e-3):
  1. LN1 over x[b] (row stats on DVE), normalized output transposed on PE
     to ln1T [c, n] fp32r.
  2. qT/kT = w.T @ ln1T per head-pair (fp32r); vT likewise then
     PE-transposed to v-natural [m, 65] bf16 tiles with a ones column.
  3. Per head: scoresT[m, n] = kT.T @ qT chunks; exp fused with the
     1/sqrt(C) scale on ACT (psum->sbuf bf16); attention output computed
     directly in the final interleaved layout via strided-column lhsT
     slices (out[r, 64j+d] = sum_m exp[m, 16r+j] * v[m, d]); the ones
     column yields the softmax denominator in psum column 64, applied by
     DVE reciprocal + per-partition multiply. Residual add with x rows.
  4. LN2 per 128-row block, PE-transposed to ln2T fp32r.
  5. MLP: h1T = w1.T @ ln2T (single pass over w1 streamed from HBM),
     relu+bias fused in the ACT evacuation; ff = h1T.T @ w2 with a K=1
     ones-matmul folding in b2; final residual add and DMA out.
"""

import os
import sys
from contextlib import ExitStack

for _p in ("/opt/trn_rl_repo", "/root/.axon_site/_ro/trn_rl_repo"):
    if os.path.isdir(_p) and _p not in sys.path:
        sys.path.insert(0, _p)

import numpy as np

import concourse.bass as bass
import concourse.tile as tile
from concourse import bacc, mybir
from concourse.bass_utils import run_bass_kernel_spmd
from concourse.masks import make_identity

F32 = mybir.dt.float32
F32R = mybir.dt.float32r
BF16 = mybir.dt.bfloat16
AF = mybir.ActivationFunctionType
OP = mybir.AluOpType

P = 128
B, N, C = 2, 2048, 1024
H, HD = 16, 64
H4 = 4 * C
EPS = 1e-5
SCALE = 1.0 / 32.0  # 1/sqrt(C)

NH = 4            # heads per core
NPAIR = 2         # head pairs per core
ROWS = NH * P     # output rows per core (512)
NCHUNK = N // P   # 16 sequence chunks
CCH = C // P      # 8 channel chunks
HKN = H4 // P     # 32 hidden chunks

_TS = bass.ts


def _emit(nc):
    x = nc.dram_tensor("x", (N, C), F32, kind="ExternalInput").ap()
    xown = nc.dram_tensor("xown", (ROWS, C), F32, kind="ExternalInput").ap()
    wq = nc.dram_tensor("wq", (C, NH * HD), BF16, kind="ExternalInput").ap()
    wk = nc.dram_tensor("wk", (C, NH * HD), BF16, kind="ExternalInput").ap()
    wv = nc.dram_tensor("wv", (C, NH * HD), BF16, kind="ExternalInput").ap()
    qb = nc.dram_tensor("qb", (NH * HD,), F32, kind="ExternalInput").ap()
    kb = nc.dram_tensor("kb", (NH * HD,), F32, kind="ExternalInput").ap()
    vb = nc.dram_tensor("vb", (NH * HD,), F32, kind="ExternalInput").ap()
    w1 = nc.dram_tensor("w1", (C, H4), BF16, kind="ExternalInput").ap()
    b1 = nc.dram_tensor("b1", (H4,), F32, kind="ExternalInput").ap()
    w2 = nc.dram_tensor("w2", (H4, C), BF16, kind="ExternalInput").ap()
    b2 = nc.dram_tensor("b2", (C,), F32R, kind="ExternalInput").ap()
    g1 = nc.dram_tensor("g1", (C,), F32, kind="ExternalInput").ap()
    bb1 = nc.dram_tensor("bb1", (C,), F32, kind="ExternalInput").ap()
    g2 = nc.dram_tensor("g2", (C,), F32, kind="ExternalInput").ap()
    bb2 = nc.dram_tensor("bb2", (C,), F32, kind="ExternalInput").ap()
    y = nc.dram_tensor("y", (ROWS, C), F32, kind="ExternalOutput").ap()
    dbg = None
    if os.environ.get("KERNEL_DEBUG"):
        dbg = nc.dram_tensor("dbg", (ROWS, C), F32, kind="ExternalOutput").ap()

    reps = int(os.environ.get("KERNEL_REPS", "1"))
    with tile.TileContext(nc) as tc:
        for _ in range(reps):
            _body(tc, nc, x, xown, wq, wk, wv, qb, kb, vb,
                  w1, b1, w2, b2, g1, bb1, g2, bb2, y, dbg)
    return nc


def _body(tc, nc, x, xown, wq, wk, wv, qb, kb, vb,
          w1, b1, w2, b2, g1, bb1, g2, bb2, y, dbg=None):
    with ExitStack() as ctx:
        singles = ctx.enter_context(tc.tile_pool(name="singles", bufs=1))

        # --- constants -------------------------------------------------
        id_f = singles.tile([P, P], F32)
        make_identity(nc, id_f[:])
        id_r = singles.tile([P, P], F32R)
        nc.vector.tensor_copy(id_r[:], id_f[:])
        id_b = singles.tile([P, P], BF16)
        make_identity(nc, id_b[:])
        eps_t = singles.tile([P, 1], F32)
        nc.vector.memset(eps_t[:], EPS)
        ones_f = singles.tile([1, P], F32)
        nc.vector.memset(ones_f[:], 1.0)
        ones_row = singles.tile([1, P], F32R)
        nc.vector.tensor_copy(ones_row[:], ones_f[:])

        g1_sb = singles.tile([P, CCH], F32)
        nc.sync.dma_start(g1_sb[:], g1.rearrange("(k p) -> p k", p=P))
        bb1_sb = singles.tile([P, CCH], F32)
        nc.sync.dma_start(bb1_sb[:], bb1.rearrange("(k p) -> p k", p=P))
        g2_sb = singles.tile([P, CCH], F32)
        nc.sync.dma_start(g2_sb[:], g2.rearrange("(k p) -> p k", p=P))
        bb2_sb = singles.tile([P, CCH], F32)
        nc.sync.dma_start(bb2_sb[:], bb2.rearrange("(k p) -> p k", p=P))
        qb_sb = singles.tile([P, NPAIR], F32)
        nc.sync.dma_start(qb_sb[:], qb.rearrange("(pr p) -> p pr", p=P))
        kb_sb = singles.tile([P, NPAIR], F32)
        nc.sync.dma_start(kb_sb[:], kb.rearrange("(pr p) -> p pr", p=P))
        vb_sb = singles.tile([P, NPAIR], F32)
        nc.sync.dma_start(vb_sb[:], vb.rearrange("(pr p) -> p pr", p=P))
        b1_sb = singles.tile([P, HKN], F32)
        nc.sync.dma_start(b1_sb[:], b1.rearrange("(k p) -> p k", p=P))
        b2_sb = singles.tile([1, C], F32R)
        nc.sync.dma_start(b2_sb[:], b2[None, :])

        # persistent activations spanning attention+MLP
        xk = singles.tile([P, NH, C], F32)
        x2 = singles.tile([P, NH, C], F32)
        ln2T = singles.tile([P, CCH, ROWS], BF16)

        with ExitStack() as actx:
            attn = actx.enter_context(tc.tile_pool(name="attn", bufs=1))
            qT = attn.tile([P, NPAIR, N], BF16)
            kT = attn.tile([P, NPAIR, N], BF16)
            v_sb = attn.tile([P, NH, NCHUNK, HD + 1], BF16)

            # ------------- phase 1+2: LN1+transpose, QKV --------------
            with (
                tc.tile_pool(name="ph1", bufs=2) as ph1,
                tc.tile_pool(name="pp_a", bufs=3, space="PSUM") as pp_a,
            ):
                ln1T = ph1.tile([P, CCH, N], BF16, tag="ln1T", bufs=1)
                vT = ph1.tile([P, NPAIR, N], BF16, tag="vT", bufs=1)
                wq_sb = ph1.tile([P, CCH, NH * HD], BF16, tag="wq", bufs=1)
                nc.sync.dma_start(wq_sb[:], wq.rearrange("(k p) m -> p k m", p=P))
                wk_sb = ph1.tile([P, CCH, NH * HD], BF16, tag="wk", bufs=1)
                nc.sync.dma_start(wk_sb[:], wk.rearrange("(k p) m -> p k m", p=P))
                wv_sb = ph1.tile([P, CCH, NH * HD], BF16, tag="wv", bufs=1)
                nc.sync.dma_start(wv_sb[:], wv.rearrange("(k p) m -> p k m", p=P))
                for t in range(NCHUNK):
                    x_t = ph1.tile([P, C], F32, tag="xt")
                    nc.sync.dma_start(x_t[:], x[_TS(t, P), :])
                    stats = ph1.tile([P, 2, 6], F32, tag="st")
                    nc.vector.bn_stats(stats[:, 0, :], x_t[:, 0:512])
                    nc.vector.bn_stats(stats[:, 1, :], x_t[:, 512:1024])
                    mv = ph1.tile([P, 2], F32, tag="mv")
                    nc.vector.bn_aggr(mv[:], stats[:])
                    rstd = ph1.tile([P, 1], F32, tag="rs")
                    nc.scalar.activation(rstd[:], mv[:, 1:2], AF.Sqrt,
                                         bias=eps_t[:], scale=1.0)
                    nc.vector.reciprocal(rstd[:], rstd[:])
                    xn = ph1.tile([P, C], BF16, tag="xn")
                    nc.vector.tensor_scalar(
                        out=xn[:], in0=x_t[:], scalar1=mv[:, 0:1],
                        scalar2=rstd[:], op0=OP.subtract, op1=OP.mult)
                    for k in range(CCH):
                        pt = pp_a.tile([P, 512], BF16, tag="ps")
                        nc.tensor.transpose(pt[:, 0:P], xn[:, _TS(k, P)],
                                            id_b[:])
                        if k % 2 == 0:
                            nc.vector.tensor_scalar(
                                out=ln1T[:, k, _TS(t, P)], in0=pt[:, 0:P],
                                scalar1=g1_sb[:, k:k + 1],
                                scalar2=bb1_sb[:, k:k + 1],
                                op0=OP.mult, op1=OP.add)
                        else:
                            nc.scalar.activation(
                                ln1T[:, k, _TS(t, P)], pt[:, 0:P],
                                AF.Identity, bias=bb1_sb[:, k:k + 1],
                                scale=g1_sb[:, k:k + 1])

                for pr in range(NPAIR):
                    for nb in range(4):
                        for wsb, bias_sb, dst in (
                            (wq_sb, qb_sb, qT), (wk_sb, kb_sb, kT)):
                            ps = pp_a.tile([P, 512], F32, tag="ps")
                            for kc in range(CCH):
                                nc.tensor.matmul(
                                    ps[:], wsb[:, kc, _TS(pr, P)],
                                    ln1T[:, kc, _TS(nb, 512)],
                                    start=(kc == 0), stop=(kc == CCH - 1))
                            nc.vector.tensor_scalar(
                                out=dst[:, pr, _TS(nb, 512)], in0=ps[:],
                                scalar1=bias_sb[:, pr:pr + 1], scalar2=None,
                                op0=OP.add)
                        ps = pp_a.tile([P, 512], F32, tag="ps")
                        for kc in range(CCH):
                            nc.tensor.matmul(
                                ps[:], wv_sb[:, kc, _TS(pr, P)],
                                ln1T[:, kc, _TS(nb, 512)],
                                start=(kc == 0), stop=(kc == CCH - 1))
                        nc.scalar.activation(
                            vT[:, pr, _TS(nb, 512)], ps[:], AF.Identity,
                            bias=vb_sb[:, pr:pr + 1], scale=1.0)

                nc.vector.memset(v_sb[:, :, :, HD:HD + 1], 1.0)
                for h in range(NH):
                    pr, dp = h // 2, (h % 2) * HD
                    for mc in range(NCHUNK):
                        pv = pp_a.tile([P, 1024], BF16, tag="ps")
                        nc.tensor.transpose(
                            pv[:, 0:HD], vT[dp:dp + HD, pr, _TS(mc, P)],
                            id_b[dp:dp + HD, dp:dp + HD])
                        nc.vector.tensor_copy(v_sb[:, h, mc, 0:HD],
                                              pv[:, 0:HD])

            nc.sync.dma_start(xk[:], xown.rearrange("(h p) c -> p h c", p=P))

            # ------------- phase 3: attention per head ----------------
            with (
                tc.tile_pool(name="ph3", bufs=2) as ph3,
                tc.tile_pool(name="pp_s", bufs=2, space="PSUM") as pp_s,
                tc.tile_pool(name="pp_o", bufs=4, space="PSUM") as pp_o,
            ):
                def attn_scores(h):
                    pr, dp = h // 2, (h % 2) * HD
                    lhss = []
                    for hf in range(2):
                        expT = ph3.tile([P, 8, N], BF16, tag="expT", bufs=3,
                                        name=f"expT{h}_{hf}")
                        for mc8 in range(8):
                            mc = hf * 8 + mc8
                            for nb in range(2):
                                pss = pp_s.tile([P, 1024], F32, tag="ss",
                                                name=f"pss{h}_{mc}_{nb}")
                                for nb2 in range(2):
                                    nc.tensor.matmul(
                                        pss[:, _TS(nb2, 512)],
                                        kT[dp:dp + HD, pr, _TS(mc, P)],
                                        qT[dp:dp + HD, pr,
                                           _TS(nb * 2 + nb2, 512)],
                                        start=True, stop=True)
                                nc.scalar.activation(
                                    expT[:, mc8, _TS(nb, 1024)], pss[:],
                                    AF.Exp, scale=SCALE)
                        lhss.append(
                            expT.rearrange("p c (r m) -> p c m r", m=16))
                    return lhss

                def attn_out(h, lhss):
                    # each j owns a contiguous 16-matmul accumulation group:
                    # start=True clears has_written bank-wide, so groups in
                    # a shared bank must not interleave
                    pso = [pp_o.tile([P, 4, HD + 1], F32, tag="oo", bufs=4,
                                     name=f"pso{h}_{q}") for q in range(4)]
                    for j in range(16):
                        for mc in range(16):
                            nc.tensor.matmul(
                                pso[j // 4][:, j % 4, :],
                                lhss[mc // 8][:, mc % 8, j, :],
                                v_sb[:, h, mc, :],
                                start=(mc == 0), stop=(mc == 15))
                    return pso

                def attn_post(h, pso):
                    for j in range(16):
                        rden = ph3.tile([P, 1], F32, tag="rden",
                                        name=f"rden{h}_{j}")
                        nc.vector.reciprocal(
                            rden[:], pso[j // 4][:, j % 4, HD:HD + 1])
                        nc.vector.tensor_scalar(
                            out=x2[:, h, _TS(j, HD)],
                            in0=pso[j // 4][:, j % 4, 0:HD],
                            scalar1=rden[:], scalar2=None, op0=OP.mult)
                    nc.vector.tensor_add(x2[:, h, :], x2[:, h, :],
                                         xk[:, h, :])

                    # LN2 for this block + transpose
                    stats2 = ph3.tile([P, 2, 6], F32, tag="st2")
                    nc.vector.bn_stats(stats2[:, 0, :], x2[:, h, 0:512])
                    nc.vector.bn_stats(stats2[:, 1, :], x2[:, h, 512:1024])
                    mv2 = ph3.tile([P, 2], F32, tag="mv2")
                    nc.vector.bn_aggr(mv2[:], stats2[:])
                    rstd2 = ph3.tile([P, 1], F32, tag="rs2")
                    nc.scalar.activation(rstd2[:], mv2[:, 1:2], AF.Sqrt,
                                         bias=eps_t[:], scale=1.0)
                    nc.vector.reciprocal(rstd2[:], rstd2[:])
                    xn2 = ph3.tile([P, C], BF16, tag="xn2")
                    nc.vector.tensor_scalar(
                        out=xn2[:], in0=x2[:, h, :], scalar1=mv2[:, 0:1],
                        scalar2=rstd2[:], op0=OP.subtract, op1=OP.mult)
                    for k in range(CCH):
                        pt = pp_s.tile([P, 1024], BF16, tag="ss",
                                       name=f"pt2{h}_{k}")
                        nc.tensor.transpose(pt[:, 0:P], xn2[:, _TS(k, P)],
                                            id_b[:])
                        if k % 2 == 0:
                            nc.vector.tensor_scalar(
                                out=ln2T[:, k, _TS(h, P)], in0=pt[:, 0:P],
                                scalar1=g2_sb[:, k:k + 1],
                                scalar2=bb2_sb[:, k:k + 1],
                                op0=OP.mult, op1=OP.add)
                        else:
                            nc.scalar.activation(
                                ln2T[:, k, _TS(h, P)], pt[:, 0:P],
                                AF.Identity, bias=bb2_sb[:, k:k + 1],
                                scale=g2_sb[:, k:k + 1])

                # software-pipelined: scores/exp of head h+1 overlap the
                # attention-output matmuls of head h (expT slots: bufs=3)
                lh = {0: attn_scores(0)}
                for h in range(1, NH):
                    lh[h] = attn_scores(h)
                    pso = attn_out(h - 1, lh.pop(h - 1))
                    attn_post(h - 1, pso)
                pso = attn_out(NH - 1, lh.pop(NH - 1))
                attn_post(NH - 1, pso)

        # ---------------- phase 4+5: MLP ------------------------------
        with ExitStack() as mctx:
            mlp = mctx.enter_context(tc.tile_pool(name="mlp", bufs=1))
            h1T = mlp.tile([P, HKN, ROWS], BF16)
            w2sb = mlp.tile([P, HKN, C], BF16)
            nc.sync.dma_start(w2sb[:], w2.rearrange("(k p) c -> p k c", p=P))
            with (
                tc.tile_pool(name="w1p", bufs=6) as w1p,
                tc.tile_pool(name="pp_m", bufs=2, space="PSUM") as pp_m,
            ):
                w1r = w1.rearrange("(k p) hh -> p k hh", p=P)
                for hk in range(HKN):
                    w1c = w1p.tile([P, CCH, P], BF16, tag="w1c")
                    nc.sync.dma_start(w1c[:], w1r[:, :, _TS(hk, P)])
                    psh = pp_m.tile([P, ROWS], F32, tag="mm")
                    for kc in range(CCH):
                        nc.tensor.matmul(
                            psh[:], w1c[:, kc, :], ln2T[:, kc, :],
                            start=(kc == 0), stop=(kc == CCH - 1))
                    nc.scalar.activation(
                        h1T[:, hk, :], psh[:], AF.Relu,
                        bias=b1_sb[:, hk:hk + 1], scale=1.0)

            with (
                tc.tile_pool(name="ph5", bufs=3) as ph5,
                tc.tile_pool(name="pp_f", bufs=8, space="PSUM") as pp_f,
            ):
                psf = [pp_f.tile([P, 512], F32, tag="ff", bufs=8,
                                 name=f"psf{q}") for q in range(8)]
                for q in range(8):
                    nc.tensor.matmul(
                        psf[q][:], ones_row[:], b2_sb[0:1, _TS(q % 2, 512)],
                        start=True, stop=False)
                for hk in range(HKN):
                    for j in range(4):
                        for cg in range(2):
                            nc.tensor.matmul(
                                psf[j * 2 + cg][:],
                                h1T[:, hk, _TS(j, P)],
                                w2sb[:, hk, _TS(cg, 512)],
                                start=False, stop=(hk == HKN - 1))
                for j in range(4):
                    for cg in range(2):
                        y_sb = ph5.tile([P, 512], F32, tag="ysb")
                        nc.vector.tensor_add(
                            y_sb[:], psf[j * 2 + cg][:],
                            x2[:, j, _TS(cg, 512)])
                        nc.sync.dma_start(y[_TS(j, P), _TS(cg, 512)],
                                          y_sb[:])


_NC_CACHE = {}


def _get_nc():
    key = os.environ.get("KERNEL_REPS", "1")
    if key not in _NC_CACHE:
        nc = bacc.Bacc("TRN2", target_bir_lowering=False, debug=False,
                       num_devices=8)
        _emit(nc)
        nc.compile()
        _NC_CACHE[key] = nc
    return _NC_CACHE[key]


def make_in_maps(x, qkv_w, qkv_b, w1, b1, w2, b2, ln1_g, ln1_b, ln2_g, ln2_b):
    import ml_dtypes
    x = np.asarray(x, dtype=np.float32)
    qkv_w = np.asarray(qkv_w, dtype=np.float32)
    qkv_b = np.asarray(qkv_b, dtype=np.float32)
    w1 = np.ascontiguousarray(
        np.asarray(w1, dtype=np.float32).astype(ml_dtypes.bfloat16))
    b1 = np.asarray(b1, dtype=np.float32)
    w2 = np.ascontiguousarray(
        np.asarray(w2, dtype=np.float32).astype(ml_dtypes.bfloat16))
    b2 = np.asarray(b2, dtype=np.float32)
    in_maps = []
    for core in range(8):
        b, g = divmod(core, 4)
        cs = slice(256 * g, 256 * (g + 1))
        in_maps.append({
            "x": np.ascontiguousarray(x[b]),
            "xown": np.ascontiguousarray(x[b, 512 * g:512 * (g + 1)]),
            "wq": np.ascontiguousarray(
                qkv_w[:, cs].astype(ml_dtypes.bfloat16)),
            "wk": np.ascontiguousarray(
                qkv_w[:, C:2 * C][:, cs].astype(ml_dtypes.bfloat16)),
            "wv": np.ascontiguousarray(
                qkv_w[:, 2 * C:][:, cs].astype(ml_dtypes.bfloat16)),
            "qb": np.ascontiguousarray(qkv_b[cs]),
            "kb": np.ascontiguousarray(qkv_b[C:2 * C][cs]),
            "vb": np.ascontiguousarray(qkv_b[2 * C:][cs]),
            "w1": w1, "b1": b1, "w2": w2, "b2": b2,
            "g1": np.asarray(ln1_g, np.float32),
            "bb1": np.asarray(ln1_b, np.float32),
            "g2": np.asarray(ln2_g, np.float32),
            "bb2": np.asarray(ln2_b, np.float32),
        })
    return in_maps


def kernel(x, qkv_w, qkv_b, w1, b1, w2, b2, ln1_g, ln1_b, ln2_g, ln2_b):
    nc = _get_nc()
    in_maps = make_in_maps(x, qkv_w, qkv_b, w1, b1, w2, b2,
                           ln1_g, ln1_b, ln2_g, ln2_b)
    res = run_bass_kernel_spmd(nc, in_maps, core_ids=list(range(8)))
    out = np.empty((B, N, C), dtype=np.float32)
    for core in range(8):
        b, g = divmod(core, 4)
        out[b, 512 * g:512 * (g + 1)] = res.results[core]["y"]
    return out



# revision 6
# speedup vs baseline: 1.3333x; 1.3333x over previous
"""Trainium2 Bass kernel for a transformer block (dense_transformer).

Reference computation (B=2, N=2048, C=1024, 16 heads, head_dim=64):
    x = x + attn(LN1(x))            # attn WITHOUT output projection; heads
                                    # interleaved by the faithful reshape
    out = x + MLP(LN2(x))           # MLP = relu(x@w1+b1)@w2+b2

Sharding: 8 cores; core c handles batch b=c//4 and heads 4g..4g+3 (g=c%4).
Because the reference reshapes [B,H,N,hd]->[B,N,C] without transposing
heads back, head h's attention output occupies output rows
[128h, 128h+128) of batch b: out[b, 128h+r, 64m+d] = attn_h[16r+m, d].
So a (batch, 4-head) shard produces a contiguous [512, 1024] output slab
and the whole residual+MLP for those rows is local to the core.

Performance notes (vs the first working version, 609us):
  - The PE HAM clock gate halves the PE clock whenever the engine idles
    >~3.4us, and the old kernel oscillated cold/warm all run.  A warmup
    matmul burst + interleaving QKV matmuls into the LN1 chunk loop +
    prefetching w2 during attention keeps the PE at 2.4 GHz.
  - All four LayerNorm affine params and the v/qkv bias are folded
    host-side: ln1_g into wq/wk/wv rows, ln1_b into the qkv bias,
    ln2_g into w1 rows, ln2_b into b1, and the v-bias directly into the
    residual input xk (softmax rows sum to 1, so attn(v + 1*vb) =
    attn(v) + vb broadcast).  PSUM evacuations become pure copies and
    batch 4..8 transposes per instruction.
  - v is computed directly in natural [seq, d] layout
    (lhsT=ln1T chunk, rhs=wv), killing 64 PE transposes per core.
  - exp on ACT is the attention-phase floor (~121us/core); everything
    else in that phase is kept off ACT.
"""

import os
import sys
from contextlib import ExitStack

for _p in ("/opt/trn_rl_repo", "/root/.axon_site/_ro/trn_rl_repo"):
    if os.path.isdir(_p) and _p not in sys.path:
        sys.path.insert(0, _p)

import numpy as np

import concourse.bass as bass
import concourse.tile as tile
from concourse import bacc, mybir
from concourse.bass_utils import run_bass_kernel_spmd
from concourse.masks import make_identity

F32 = mybir.dt.float32
F32R = mybir.dt.float32r
BF16 = mybir.dt.bfloat16
AF = mybir.ActivationFunctionType
OP = mybir.AluOpType

P = 128
B, N, C = 2, 2048, 1024
H, HD = 16, 64
H4 = 4 * C
EPS = 1e-5
SCALE = 1.0 / 32.0  # 1/sqrt(C)

NH = 4            # heads per core
NPAIR = 2         # head pairs per core
ROWS = NH * P     # output rows per core (512)
NCHUNK = N // P   # 16 sequence chunks
CCH = C // P      # 8 channel chunks
HKN = H4 // P     # 32 hidden chunks

_TS = bass.ts


def _emit(nc):
    x = nc.dram_tensor("x", (N, C), F32, kind="ExternalInput").ap()
    xown = nc.dram_tensor("xown", (ROWS, C), F32, kind="ExternalInput").ap()
    wq = nc.dram_tensor("wq", (C, NH * HD), BF16, kind="ExternalInput").ap()
    wk = nc.dram_tensor("wk", (C, NH * HD), BF16, kind="ExternalInput").ap()
    wv = nc.dram_tensor("wv", (C, NH * HD), BF16, kind="ExternalInput").ap()
    qb = nc.dram_tensor("qb", (NH * HD,), F32, kind="ExternalInput").ap()
    kb = nc.dram_tensor("kb", (NH * HD,), F32, kind="ExternalInput").ap()
    w1 = nc.dram_tensor("w1", (C, H4), BF16, kind="ExternalInput").ap()
    b1 = nc.dram_tensor("b1", (H4,), F32, kind="ExternalInput").ap()
    w2 = nc.dram_tensor("w2", (H4, C), BF16, kind="ExternalInput").ap()
    b2 = nc.dram_tensor("b2", (C,), F32R, kind="ExternalInput").ap()
    y = nc.dram_tensor("y", (ROWS, C), F32, kind="ExternalOutput").ap()

    reps = int(os.environ.get("KERNEL_REPS", "1"))
    with tile.TileContext(nc) as tc:
        for _ in range(reps):
            _body(tc, nc, x, xown, wq, wk, wv, qb, kb, w1, b1, w2, b2, y)
    return nc


def _body(tc, nc, x, xown, wq, wk, wv, qb, kb, w1, b1, w2, b2, y):
    with ExitStack() as ctx:
        singles = ctx.enter_context(tc.tile_pool(name="singles", bufs=1))

        # --- constants -------------------------------------------------
        id_b = singles.tile([P, P], BF16)
        make_identity(nc, id_b[:])
        eps_t = singles.tile([P, 1], F32)
        nc.vector.memset(eps_t[:], EPS)
        ones_f = singles.tile([1, P], F32)
        nc.vector.memset(ones_f[:], 1.0)
        ones_row = singles.tile([1, P], F32R)
        nc.vector.tensor_copy(ones_row[:], ones_f[:])

        qb_sb = singles.tile([P, NPAIR], F32)
        nc.sync.dma_start(qb_sb[:], qb.rearrange("(pr p) -> p pr", p=P))
        kb_sb = singles.tile([P, NPAIR], F32)
        nc.sync.dma_start(kb_sb[:], kb.rearrange("(pr p) -> p pr", p=P))
        b1_sb = singles.tile([P, HKN], F32)
        nc.sync.dma_start(b1_sb[:], b1.rearrange("(k p) -> p k", p=P))
        b2_sb = singles.tile([1, C], F32R)
        nc.sync.dma_start(b2_sb[:], b2[None, :])

        # persistent activations spanning attention+MLP
        xk = singles.tile([P, NH, C], F32)
        x2 = singles.tile([P, NH, C], F32)
        ln2T = singles.tile([P, CCH, ROWS], BF16)

        # --- HAM warmup: ~18 back-to-back matmuls (~5us) so the PE
        # clock is at 2.4GHz by the time real matmuls start; they run
        # while the first x chunks stream in.
        with (
            tc.tile_pool(name="warm", bufs=1) as wp,
            tc.tile_pool(name="wpp", bufs=1, space="PSUM") as wpp,
        ):
            wsrc = wp.tile([P, 512], BF16)
            nc.vector.memset(wsrc[:], 0.0)
            wps = wpp.tile([P, 512], F32)
            for _ in range(18):
                nc.tensor.matmul(wps[:], id_b[:], wsrc[:],
                                 start=True, stop=True)

        with ExitStack() as actx:
            attn = actx.enter_context(tc.tile_pool(name="attn", bufs=1))
            qT = attn.tile([P, NPAIR, N], BF16)
            kT = attn.tile([P, NPAIR, N], BF16)
            v_sb = attn.tile([P, NH, NCHUNK, HD + 1], BF16)

            # ------------- phase 1+2: LN1+transpose, QKV --------------
            with (
                tc.tile_pool(name="ph1", bufs=2) as ph1,
                tc.tile_pool(name="pp_a", bufs=4, space="PSUM") as pp_a,
            ):
                ln1T = ph1.tile([P, CCH, N], BF16, tag="ln1T", bufs=1)
                wq_sb = ph1.tile([P, CCH, NH * HD], BF16, tag="wq", bufs=1)
                nc.sync.dma_start(wq_sb[:], wq.rearrange("(k p) m -> p k m", p=P))
                wk_sb = ph1.tile([P, CCH, NH * HD], BF16, tag="wk", bufs=1)
                nc.sync.dma_start(wk_sb[:], wk.rearrange("(k p) m -> p k m", p=P))
                wv_sb = ph1.tile([P, CCH, NH * HD], BF16, tag="wv", bufs=1)
                nc.sync.dma_start(wv_sb[:], wv.rearrange("(k p) m -> p k m", p=P))
                nc.vector.memset(v_sb[:, :, :, HD:HD + 1], 1.0)

                def qk_block(nb):
                    # q/k for seq block [512*nb, 512*nb+512) of all 4 heads
                    for pr in range(NPAIR):
                        for wsb, bias_sb, dst in (
                            (wq_sb, qb_sb, qT), (wk_sb, kb_sb, kT)):
                            ps = pp_a.tile([P, 512], F32, tag="ps", bufs=3,
                                           name=f"qk{nb}_{pr}_{dst is kT}")
                            for kc in range(CCH):
                                nc.tensor.matmul(
                                    ps[:], wsb[:, kc, _TS(pr, P)],
                                    ln1T[:, kc, _TS(nb, 512)],
                                    start=(kc == 0), stop=(kc == CCH - 1))
                            nc.scalar.activation(
                                dst[:, pr, _TS(nb, 512)], ps[:], AF.Identity,
                                bias=bias_sb[:, pr:pr + 1], scale=1.0)

                for t in range(NCHUNK):
                    x_t = ph1.tile([P, C], F32, tag="xt")
                    nc.sync.dma_start(x_t[:], x[_TS(t, P), :])
                    stats = ph1.tile([P, 2, 6], F32, tag="st")
                    nc.vector.bn_stats(stats[:, 0, :], x_t[:, 0:512])
                    nc.vector.bn_stats(stats[:, 1, :], x_t[:, 512:1024])
                    mv = ph1.tile([P, 2], F32, tag="mv")
                    nc.vector.bn_aggr(mv[:], stats[:])
                    rstd = ph1.tile([P, 1], F32, tag="rs")
                    nc.scalar.activation(rstd[:], mv[:, 1:2], AF.Sqrt,
                                         bias=eps_t[:], scale=1.0)
                    nc.vector.reciprocal(rstd[:], rstd[:])
                    nmr = ph1.tile([P, 1], F32, tag="nm")
                    nc.vector.tensor_scalar(
                        out=nmr[:], in0=mv[:, 0:1], scalar1=rstd[:],
                        scalar2=-1.0, op0=OP.mult, op1=OP.mult)
                    xn = ph1.tile([P, C], BF16, tag="xn")
                    nc.scalar.activation(xn[:], x_t[:], AF.Identity,
                                         bias=nmr[:], scale=rstd[:])
                    # transpose xn -> ln1T, 4 chunks per psum bank, one
                    # pure-copy evacuation per bank (LN affine is folded
                    # into the weights host-side)
                    for half in range(2):
                        pt = pp_a.tile([P, 4, P], BF16, tag="pt", bufs=2,
                                       name=f"pt{t}_{half}")
                        for i in range(4):
                            k = half * 4 + i
                            nc.tensor.transpose(pt[:, i, :], xn[:, _TS(k, P)],
                                                id_b[:])
                        nc.vector.tensor_copy(
                            ln1T[:, half * 4:half * 4 + 4, _TS(t, P)], pt[:])
                    # v for this seq chunk, directly in natural layout:
                    # v[n, d] = (ln1T chunk).T @ wv
                    v_ps = pp_a.tile([P, NH * HD], F32, tag="vps", bufs=2,
                                     name=f"v{t}")
                    for kc in range(CCH):
                        nc.tensor.matmul(
                            v_ps[:], ln1T[:, kc, _TS(t, P)], wv_sb[:, kc, :],
                            start=(kc == 0), stop=(kc == CCH - 1))
                    nc.scalar.activation(
                        v_sb[:, :, t, 0:HD],
                        v_ps.rearrange("p (h d) -> p h d", d=HD),
                        AF.Identity, scale=1.0)
                    if t % 4 == 3:
                        qk_block(t // 4)

            # w2 first-half prefetch + xk load run during attention
            # (second half loads during MLP1, once expT buffers free up)
            w2r = w2.rearrange("(k p) c -> p k c", p=P)
            w2p = ctx.enter_context(
                tc.tile_pool(name="w2p", bufs=1, side="right"))
            w2sb0 = w2p.tile([P, HKN, 512], BF16)
            nc.sync.dma_start(w2sb0[:], w2r[:, :, 0:512])
            nc.sync.dma_start(xk[:], xown.rearrange("(h p) c -> p h c", p=P))

            # ------------- phase 3: attention per head ----------------
            with (
                tc.tile_pool(name="ph3", bufs=2) as ph3,
                tc.tile_pool(name="pp_s", bufs=2, space="PSUM") as pp_s,
                tc.tile_pool(name="pp_o", bufs=4, space="PSUM") as pp_o,
            ):
                def attn_scores(h):
                    pr, dp = h // 2, (h % 2) * HD
                    lhss = []
                    for hf in range(2):
                        expT = ph3.tile([P, 8, N], BF16, tag="expT", bufs=3,
                                        name=f"expT{h}_{hf}")
                        for mc8 in range(8):
                            mc = hf * 8 + mc8
                            for nb in range(2):
                                pss = pp_s.tile([P, 1024], F32, tag="ss",
                                                name=f"pss{h}_{mc}_{nb}")
                                for nb2 in range(2):
                                    nc.tensor.matmul(
                                        pss[:, _TS(nb2, 512)],
                                        kT[dp:dp + HD, pr, _TS(mc, P)],
                                        qT[dp:dp + HD, pr,
                                           _TS(nb * 2 + nb2, 512)],
                                        start=True, stop=True)
                                nc.scalar.activation(
                                    expT[:, mc8, _TS(nb, 1024)], pss[:],
                                    AF.Exp, scale=SCALE)
                        lhss.append(
                            expT.rearrange("p c (r m) -> p c m r", m=16))
                    return lhss

                def attn_out(h, lhss):
                    # each j owns a contiguous 16-matmul accumulation group:
                    # start=True clears has_written bank-wide, so groups in
                    # a shared bank must not interleave
                    pso = [pp_o.tile([P, 4, HD + 1], F32, tag="oo", bufs=4,
                                     name=f"pso{h}_{q}") for q in range(4)]
                    for j in range(16):
                        for mc in range(16):
                            nc.tensor.matmul(
                                pso[j // 4][:, j % 4, :],
                                lhss[mc // 8][:, mc % 8, j, :],
                                v_sb[:, h, mc, :],
                                start=(mc == 0), stop=(mc == 15))
                    return pso

                def attn_post(h, pso):
                    for j in range(16):
                        rden = ph3.tile([P, 1], F32, tag="rden",
                                        name=f"rden{h}_{j}")
                        nc.vector.reciprocal(
                            rden[:], pso[j // 4][:, j % 4, HD:HD + 1])
                        nc.vector.tensor_scalar(
                            out=x2[:, h, _TS(j, HD)],
                            in0=pso[j // 4][:, j % 4, 0:HD],
                            scalar1=rden[:], scalar2=None, op0=OP.mult)
                    nc.vector.tensor_add(x2[:, h, :], x2[:, h, :],
                                         xk[:, h, :])

                    # LN2 for this block + transpose (pure copy out; LN2
                    # affine folded into w1/b1 host-side)
                    stats2 = ph3.tile([P, 2, 6], F32, tag="st2")
                    nc.vector.bn_stats(stats2[:, 0, :], x2[:, h, 0:512])
                    nc.vector.bn_stats(stats2[:, 1, :], x2[:, h, 512:1024])
                    mv2 = ph3.tile([P, 2], F32, tag="mv2")
                    nc.vector.bn_aggr(mv2[:], stats2[:])
                    rstd2 = ph3.tile([P, 1], F32, tag="rs2")
                    nc.scalar.activation(rstd2[:], mv2[:, 1:2], AF.Sqrt,
                                         bias=eps_t[:], scale=1.0)
                    nc.vector.reciprocal(rstd2[:], rstd2[:])
                    xn2 = ph3.tile([P, C], BF16, tag="xn2")
                    nc.vector.tensor_scalar(
                        out=xn2[:], in0=x2[:, h, :], scalar1=mv2[:, 0:1],
                        scalar2=rstd2[:], op0=OP.subtract, op1=OP.mult)
                    pt2 = pp_s.tile([P, 1024], BF16, tag="ss",
                                    name=f"pt2{h}")
                    pt2v = pt2.rearrange("p (k n) -> p k n", n=P)
                    for k in range(CCH):
                        nc.tensor.transpose(pt2v[:, k, :], xn2[:, _TS(k, P)],
                                            id_b[:])
                    nc.vector.tensor_copy(ln2T[:, :, _TS(h, P)], pt2v[:])

                # software-pipelined: scores/exp of head h+1 overlap the
                # attention-output matmuls of head h (expT slots: bufs=3)
                lh = {0: attn_scores(0)}
                for h in range(1, NH):
                    lh[h] = attn_scores(h)
                    pso = attn_out(h - 1, lh.pop(h - 1))
                    attn_post(h - 1, pso)
                pso = attn_out(NH - 1, lh.pop(NH - 1))
                attn_post(NH - 1, pso)

        # ---------------- phase 4+5: MLP ------------------------------
        with ExitStack() as mctx:
            mlp = mctx.enter_context(tc.tile_pool(name="mlp", bufs=1))
            h1T = mlp.tile([P, HKN, ROWS], BF16)
            w2sb1 = mlp.tile([P, HKN, 512], BF16)
            nc.sync.dma_start(w2sb1[:], w2r[:, :, 512:1024])
            with (
                tc.tile_pool(name="w1p", bufs=6) as w1p,
                tc.tile_pool(name="pp_m", bufs=2, space="PSUM") as pp_m,
            ):
                w1r = w1.rearrange("(k p) hh -> p k hh", p=P)
                for hk in range(HKN):
                    w1c = w1p.tile([P, CCH, P], BF16, tag="w1c")
                    nc.sync.dma_start(w1c[:], w1r[:, :, _TS(hk, P)])
                    psh = pp_m.tile([P, ROWS], F32, tag="mm")
                    for kc in range(CCH):
                        nc.tensor.matmul(
                            psh[:], w1c[:, kc, :], ln2T[:, kc, :],
                            start=(kc == 0), stop=(kc == CCH - 1))
                    nc.scalar.activation(
                        h1T[:, hk, :], psh[:], AF.Relu,
                        bias=b1_sb[:, hk:hk + 1], scale=1.0)

            with (
                tc.tile_pool(name="ph5", bufs=3) as ph5,
                tc.tile_pool(name="pp_f", bufs=8, space="PSUM") as pp_f,
            ):
                psf = [pp_f.tile([P, 512], F32, tag="ff", bufs=8,
                                 name=f"psf{q}") for q in range(8)]
                for q in range(8):
                    nc.tensor.matmul(
                        psf[q][:], ones_row[:], b2_sb[0:1, _TS(q // 4, 512)],
                        start=True, stop=False)
                for cg, w2half in ((0, w2sb0), (1, w2sb1)):
                    for hk in range(HKN):
                        for j in range(4):
                            nc.tensor.matmul(
                                psf[cg * 4 + j][:],
                                h1T[:, hk, _TS(j, P)],
                                w2half[:, hk, :],
                                start=False, stop=(hk == HKN - 1))
                for cg in range(2):
                    for j in range(4):
                        y_sb = ph5.tile([P, 512], F32, tag="ysb")
                        nc.vector.tensor_add(
                            y_sb[:], psf[cg * 4 + j][:],
                            x2[:, j, _TS(cg, 512)])
                        nc.sync.dma_start(y[_TS(j, P), _TS(cg, 512)],
                                          y_sb[:])


_NC_CACHE = {}


def _get_nc():
    key = os.environ.get("KERNEL_REPS", "1")
    if key not in _NC_CACHE:
        nc = bacc.Bacc("TRN2", target_bir_lowering=False, debug=False,
                       num_devices=8)
        _emit(nc)
        nc.compile()
        _NC_CACHE[key] = nc
    return _NC_CACHE[key]


def make_in_maps(x, qkv_w, qkv_b, w1, b1, w2, b2, ln1_g, ln1_b, ln2_g, ln2_b):
    import ml_dtypes
    x = np.asarray(x, dtype=np.float32)
    qkv_w = np.asarray(qkv_w, dtype=np.float32)
    qkv_b = np.asarray(qkv_b, dtype=np.float32)
    w1 = np.asarray(w1, dtype=np.float32)
    b1 = np.asarray(b1, dtype=np.float32)
    w2 = np.asarray(w2, dtype=np.float32)
    b2 = np.asarray(b2, dtype=np.float32)
    g1 = np.asarray(ln1_g, np.float32)
    bb1 = np.asarray(ln1_b, np.float32)
    g2 = np.asarray(ln2_g, np.float32)
    bb2 = np.asarray(ln2_b, np.float32)

    # Fold LN affine transforms into the downstream weights:
    #   qkv(LN1(x)) = (core1(x) * g1 + bb1) @ W + b
    #               = core1(x) @ (g1[:,None]*W) + (bb1 @ W + b)
    # and likewise LN2 into w1/b1.  The kernel then computes only the
    # core (x-mu)*rstd normalization on-chip.
    qkv_w_eff = g1[:, None] * qkv_w
    qkv_b_eff = qkv_b + bb1 @ qkv_w
    w1_eff = np.ascontiguousarray(
        (g2[:, None] * w1).astype(ml_dtypes.bfloat16))
    b1_eff = b1 + bb2 @ w1
    w2_bf = np.ascontiguousarray(w2.astype(ml_dtypes.bfloat16))

    vb_full = qkv_b_eff[2 * C:]
    in_maps = []
    for core in range(8):
        b, g = divmod(core, 4)
        cs = slice(256 * g, 256 * (g + 1))
        # Fold the v-bias into the residual input: softmax rows sum to 1,
        # so attention(v + 1*vb) = attention(v) + vb broadcast over rows.
        # In the interleaved output layout head h's vb tiles 16x along
        # the channels of its 128-row block.
        xown = x[b, 512 * g:512 * (g + 1)].copy()
        vb_core = vb_full[cs]
        for hl in range(NH):
            pat = np.tile(vb_core[64 * hl:64 * (hl + 1)], 16)
            xown[128 * hl:128 * (hl + 1), :] += pat[None, :]
        in_maps.append({
            "x": np.ascontiguousarray(x[b]),
            "xown": np.ascontiguousarray(xown),
            "wq": np.ascontiguousarray(
                qkv_w_eff[:, cs].astype(ml_dtypes.bfloat16)),
            "wk": np.ascontiguousarray(
                qkv_w_eff[:, C:2 * C][:, cs].astype(ml_dtypes.bfloat16)),
            "wv": np.ascontiguousarray(
                qkv_w_eff[:, 2 * C:][:, cs].astype(ml_dtypes.bfloat16)),
            "qb": np.ascontiguousarray(qkv_b_eff[cs]),
            "kb": np.ascontiguousarray(qkv_b_eff[C:2 * C][cs]),
            "w1": w1_eff, "b1": b1_eff, "w2": w2_bf, "b2": b2,
        })
    return in_maps


def kernel(x, qkv_w, qkv_b, w1, b1, w2, b2, ln1_g, ln1_b, ln2_g, ln2_b):
    nc = _get_nc()
    in_maps = make_in_maps(x, qkv_w, qkv_b, w1, b1, w2, b2,
                           ln1_g, ln1_b, ln2_g, ln2_b)
    res = run_bass_kernel_spmd(nc, in_maps, core_ids=list(range(8)))
    out = np.empty((B, N, C), dtype=np.float32)
    for core in range(8):
        b, g = divmod(core, 4)
        out[b, 512 * g:512 * (g + 1)] = res.results[core]["y"]
    return out


# revision 12
# speedup vs baseline: 1.3463x; 1.0098x over previous
"""Trainium2 Bass kernel for a transformer block (dense_transformer).

Reference computation (B=2, N=2048, C=1024, 16 heads, head_dim=64):
    x = x + attn(LN1(x))            # attn WITHOUT output projection; heads
                                    # interleaved by the faithful reshape
    out = x + MLP(LN2(x))           # MLP = relu(x@w1+b1)@w2+b2

Sharding: 8 cores; core c handles batch b=c//4 and heads 4g..4g+3 (g=c%4).
Because the reference reshapes [B,H,N,hd]->[B,N,C] without transposing
heads back, head h's attention output occupies output rows
[128h, 128h+128) of batch b: out[b, 128h+r, 64m+d] = attn_h[16r+m, d].
So a (batch, 4-head) shard produces a contiguous [512, 1024] output slab
and the whole residual+MLP for those rows is local to the core.

Performance notes (vs the first working version, 609us):
  - The PE HAM clock gate halves the PE clock whenever the engine idles
    >~3.4us, and the old kernel oscillated cold/warm all run.  A warmup
    matmul burst + interleaving QKV matmuls into the LN1 chunk loop +
    prefetching w2 during attention keeps the PE at 2.4 GHz.
  - All four LayerNorm affine params and the v/qkv bias are folded
    host-side: ln1_g into wq/wk/wv rows, ln1_b into the qkv bias,
    ln2_g into w1 rows, ln2_b into b1, and the v-bias directly into the
    residual input xk (softmax rows sum to 1, so attn(v + 1*vb) =
    attn(v) + vb broadcast).  PSUM evacuations become pure copies and
    batch 4..8 transposes per instruction.
  - v is computed directly in natural [seq, d] layout
    (lhsT=ln1T chunk, rhs=wv), killing 64 PE transposes per core.
  - exp on ACT is the attention-phase floor (~121us/core); everything
    else in that phase is kept off ACT.
"""

import os
import sys
from contextlib import ExitStack

for _p in ("/opt/trn_rl_repo", "/root/.axon_site/_ro/trn_rl_repo"):
    if os.path.isdir(_p) and _p not in sys.path:
        sys.path.insert(0, _p)

import numpy as np

import concourse.bass as bass
import concourse.tile as tile
from concourse import bacc, mybir
from concourse.bass_utils import run_bass_kernel_spmd
from concourse.masks import make_identity

F32 = mybir.dt.float32
F32R = mybir.dt.float32r
BF16 = mybir.dt.bfloat16
AF = mybir.ActivationFunctionType
OP = mybir.AluOpType

P = 128
B, N, C = 2, 2048, 1024
H, HD = 16, 64
H4 = 4 * C
EPS = 1e-5
SCALE = 1.0 / 32.0  # 1/sqrt(C)

NH = 4            # heads per core
NPAIR = 2         # head pairs per core
ROWS = NH * P     # output rows per core (512)
NCHUNK = N // P   # 16 sequence chunks
CCH = C // P      # 8 channel chunks
HKN = H4 // P     # 32 hidden chunks

_TS = bass.ts


def _emit(nc):
    x = nc.dram_tensor("x", (N, C), F32, kind="ExternalInput").ap()
    xown = nc.dram_tensor("xown", (ROWS, C), F32, kind="ExternalInput").ap()
    wq = nc.dram_tensor("wq", (C, NH * HD), BF16, kind="ExternalInput").ap()
    wk = nc.dram_tensor("wk", (C, NH * HD), BF16, kind="ExternalInput").ap()
    wv = nc.dram_tensor("wv", (C, NH * HD), BF16, kind="ExternalInput").ap()
    qb = nc.dram_tensor("qb", (NH * HD,), F32, kind="ExternalInput").ap()
    kb = nc.dram_tensor("kb", (NH * HD,), F32, kind="ExternalInput").ap()
    w1 = nc.dram_tensor("w1", (C, H4), BF16, kind="ExternalInput").ap()
    b1 = nc.dram_tensor("b1", (H4,), F32, kind="ExternalInput").ap()
    w2 = nc.dram_tensor("w2", (H4, C), BF16, kind="ExternalInput").ap()
    b2 = nc.dram_tensor("b2", (C,), F32R, kind="ExternalInput").ap()
    y = nc.dram_tensor("y", (ROWS, C), F32, kind="ExternalOutput").ap()

    reps = int(os.environ.get("KERNEL_REPS", "1"))
    with tile.TileContext(nc) as tc:
        for _ in range(reps):
            _body(tc, nc, x, xown, wq, wk, wv, qb, kb, w1, b1, w2, b2, y)
    return nc


def _body(tc, nc, x, xown, wq, wk, wv, qb, kb, w1, b1, w2, b2, y):
    with ExitStack() as ctx:
        singles = ctx.enter_context(tc.tile_pool(name="singles", bufs=1))

        # --- constants -------------------------------------------------
        id_b = singles.tile([P, P], BF16)
        make_identity(nc, id_b[:])
        eps_t = singles.tile([P, 1], F32)
        nc.vector.memset(eps_t[:], EPS)
        ones_f = singles.tile([1, P], F32)
        nc.vector.memset(ones_f[:], 1.0)
        ones_row = singles.tile([1, P], F32R)
        nc.vector.tensor_copy(ones_row[:], ones_f[:])

        qb_sb = singles.tile([P, NPAIR], F32)
        nc.sync.dma_start(qb_sb[:], qb.rearrange("(pr p) -> p pr", p=P))
        kb_sb = singles.tile([P, NPAIR], F32)
        nc.sync.dma_start(kb_sb[:], kb.rearrange("(pr p) -> p pr", p=P))
        b1_sb = singles.tile([P, HKN], F32)
        nc.sync.dma_start(b1_sb[:], b1.rearrange("(k p) -> p k", p=P))
        b2_sb = singles.tile([1, C], F32R)
        nc.sync.dma_start(b2_sb[:], b2[None, :])

        # persistent activations spanning attention+MLP
        xk = singles.tile([P, NH, C], F32)
        x2 = singles.tile([P, NH, C], F32)
        ln2T = singles.tile([P, CCH, ROWS], BF16)

        # --- HAM warmup: ~18 back-to-back matmuls (~5us) so the PE
        # clock is at 2.4GHz by the time real matmuls start; they run
        # while the first x chunks stream in.
        with (
            tc.tile_pool(name="warm", bufs=1) as wp,
            tc.tile_pool(name="wpp", bufs=1, space="PSUM") as wpp,
        ):
            wsrc = wp.tile([P, 512], BF16)
            nc.vector.memset(wsrc[:], 0.0)
            wps = wpp.tile([P, 512], F32)
            for _ in range(36):
                nc.tensor.matmul(wps[:], id_b[:], wsrc[:],
                                 start=True, stop=True)

        with ExitStack() as actx:
            attn = actx.enter_context(tc.tile_pool(name="attn", bufs=1))
            qT = attn.tile([P, NPAIR, N], BF16)
            kT = attn.tile([P, NPAIR, N], BF16)
            v_sb = attn.tile([P, NH, NCHUNK, HD + 1], BF16)

            # ------------- phase 1+2: LN1+transpose, QKV --------------
            with (
                tc.tile_pool(name="ph1", bufs=2) as ph1,
                tc.tile_pool(name="pp_a", bufs=4, space="PSUM") as pp_a,
            ):
                ln1T = ph1.tile([P, CCH, N], BF16, tag="ln1T", bufs=1)
                wq_sb = ph1.tile([P, CCH, NH * HD], BF16, tag="wq", bufs=1)
                nc.sync.dma_start(wq_sb[:], wq.rearrange("(k p) m -> p k m", p=P))
                wk_sb = ph1.tile([P, CCH, NH * HD], BF16, tag="wk", bufs=1)
                nc.sync.dma_start(wk_sb[:], wk.rearrange("(k p) m -> p k m", p=P))
                wv_sb = ph1.tile([P, CCH, NH * HD], BF16, tag="wv", bufs=1)
                nc.sync.dma_start(wv_sb[:], wv.rearrange("(k p) m -> p k m", p=P))
                nc.vector.memset(v_sb[:, :, :, HD:HD + 1], 1.0)

                def qk_block(nb):
                    # q/k for seq block [512*nb, 512*nb+512) of all 4 heads
                    for pr in range(NPAIR):
                        for wsb, bias_sb, dst in (
                            (wq_sb, qb_sb, qT), (wk_sb, kb_sb, kT)):
                            ps = pp_a.tile([P, 512], F32, tag="ps", bufs=3,
                                           name=f"qk{nb}_{pr}_{dst is kT}")
                            for kc in range(CCH):
                                nc.tensor.matmul(
                                    ps[:], wsb[:, kc, _TS(pr, P)],
                                    ln1T[:, kc, _TS(nb, 512)],
                                    start=(kc == 0), stop=(kc == CCH - 1))
                            nc.scalar.activation(
                                dst[:, pr, _TS(nb, 512)], ps[:], AF.Identity,
                                bias=bias_sb[:, pr:pr + 1], scale=1.0)

                for t in range(NCHUNK):
                    x_t = ph1.tile([P, C], F32, tag="xt")
                    nc.sync.dma_start(x_t[:], x[_TS(t, P), :])
                    stats = ph1.tile([P, 2, 6], F32, tag="st")
                    nc.vector.bn_stats(stats[:, 0, :], x_t[:, 0:512])
                    nc.vector.bn_stats(stats[:, 1, :], x_t[:, 512:1024])
                    mv = ph1.tile([P, 2], F32, tag="mv")
                    nc.vector.bn_aggr(mv[:], stats[:])
                    rstd = ph1.tile([P, 1], F32, tag="rs")
                    nc.scalar.activation(rstd[:], mv[:, 1:2], AF.Sqrt,
                                         bias=eps_t[:], scale=1.0)
                    nc.vector.reciprocal(rstd[:], rstd[:])
                    nmr = ph1.tile([P, 1], F32, tag="nm")
                    nc.vector.tensor_scalar(
                        out=nmr[:], in0=mv[:, 0:1], scalar1=rstd[:],
                        scalar2=-1.0, op0=OP.mult, op1=OP.mult)
                    xn = ph1.tile([P, C], BF16, tag="xn")
                    nc.scalar.activation(xn[:], x_t[:], AF.Identity,
                                         bias=nmr[:], scale=rstd[:])
                    # transpose xn -> ln1T, 4 chunks per psum bank, one
                    # pure-copy evacuation per bank (LN affine is folded
                    # into the weights host-side)
                    for half in range(2):
                        pt = pp_a.tile([P, 4, P], BF16, tag="pt", bufs=2,
                                       name=f"pt{t}_{half}")
                        for i in range(4):
                            k = half * 4 + i
                            nc.tensor.transpose(pt[:, i, :], xn[:, _TS(k, P)],
                                                id_b[:])
                        dst = ln1T[:, half * 4:half * 4 + 4, _TS(t, P)]
                        if half == 0:
                            nc.vector.tensor_copy(dst, pt[:])
                        else:
                            nc.scalar.activation(dst, pt[:], AF.Identity,
                                                 scale=1.0)
                    # v for this seq chunk, directly in natural layout:
                    # v[n, d] = (ln1T chunk).T @ wv
                    v_ps = pp_a.tile([P, NH * HD], F32, tag="vps", bufs=2,
                                     name=f"v{t}")
                    for kc in range(CCH):
                        nc.tensor.matmul(
                            v_ps[:], ln1T[:, kc, _TS(t, P)], wv_sb[:, kc, :],
                            start=(kc == 0), stop=(kc == CCH - 1))
                    nc.scalar.activation(
                        v_sb[:, :, t, 0:HD],
                        v_ps.rearrange("p (h d) -> p h d", d=HD),
                        AF.Identity, scale=1.0)
                    if t % 4 == 3:
                        qk_block(t // 4)

            # w2 first-half prefetch + xk load run during attention
            # (second half loads during MLP1, once expT buffers free up)
            w2r = w2.rearrange("(k p) c -> p k c", p=P)
            w2p = ctx.enter_context(
                tc.tile_pool(name="w2p", bufs=1, side="right"))
            w2sb0 = w2p.tile([P, HKN, 512], BF16)
            nc.sync.dma_start(w2sb0[:], w2r[:, :, 0:512])
            nc.sync.dma_start(xk[:], xown.rearrange("(h p) c -> p h c", p=P))

            # ------------- phase 3: attention per head ----------------
            with (
                tc.tile_pool(name="ph3", bufs=2) as ph3,
                tc.tile_pool(name="pp_s", bufs=2, space="PSUM") as pp_s,
                tc.tile_pool(name="pp_o", bufs=4, space="PSUM") as pp_o,
            ):
                def attn_scores_emitters(h):
                    """expT tiles + a list of 32 emit-callables, each one
                    (mc, nb) scores-matmul pair + its exp evacuation."""
                    pr, dp = h // 2, (h % 2) * HD
                    expTs = [ph3.tile([P, 8, N], BF16, tag="expT", bufs=3,
                                      name=f"expT{h}_{hf}")
                             for hf in range(2)]

                    def unit(hf, mc8, nb):
                        mc = hf * 8 + mc8
                        pss = pp_s.tile([P, 1024], F32, tag="ss",
                                        name=f"pss{h}_{mc}_{nb}")
                        for nb2 in range(2):
                            nc.tensor.matmul(
                                pss[:, _TS(nb2, 512)],
                                kT[dp:dp + HD, pr, _TS(mc, P)],
                                qT[dp:dp + HD, pr, _TS(nb * 2 + nb2, 512)],
                                start=True, stop=True)
                        nc.scalar.activation(
                            expTs[hf][:, mc8, _TS(nb, 1024)], pss[:],
                            AF.Exp, scale=SCALE)

                    ems = [lambda a=(hf, mc8, nb): unit(*a)
                           for hf in range(2)
                           for mc8 in range(8)
                           for nb in range(2)]
                    lhss = [t.rearrange("p c (r m) -> p c m r", m=16)
                            for t in expTs]
                    return ems, lhss

                def attn_out(h, lhss, ems=()):
                    # each j owns a contiguous 16-matmul accumulation group:
                    # start=True clears has_written bank-wide, so groups in
                    # a shared bank must not interleave.  The next head's
                    # scores units (ems) are emitted between j-groups so
                    # the PE stays fed while ACT streams exp.
                    pso = [pp_o.tile([P, 4, HD + 1], F32, tag="oo", bufs=4,
                                     name=f"pso{h}_{q}") for q in range(4)]
                    for j in range(16):
                        for mc in range(16):
                            nc.tensor.matmul(
                                pso[j // 4][:, j % 4, :],
                                lhss[mc // 8][:, mc % 8, j, :],
                                v_sb[:, h, mc, :],
                                start=(mc == 0), stop=(mc == 15))
                        for e in ems[2 * j:2 * j + 2]:
                            e()
                    return pso

                def attn_post(h, pso):
                    for j in range(16):
                        rden = ph3.tile([P, 1], F32, tag="rden",
                                        name=f"rden{h}_{j}")
                        nc.vector.reciprocal(
                            rden[:], pso[j // 4][:, j % 4, HD:HD + 1])
                        nc.vector.tensor_scalar(
                            out=x2[:, h, _TS(j, HD)],
                            in0=pso[j // 4][:, j % 4, 0:HD],
                            scalar1=rden[:], scalar2=None, op0=OP.mult)
                    nc.vector.tensor_add(x2[:, h, :], x2[:, h, :],
                                         xk[:, h, :])

                    # LN2 for this block + transpose (pure copy out; LN2
                    # affine folded into w1/b1 host-side)
                    stats2 = ph3.tile([P, 2, 6], F32, tag="st2")
                    nc.vector.bn_stats(stats2[:, 0, :], x2[:, h, 0:512])
                    nc.vector.bn_stats(stats2[:, 1, :], x2[:, h, 512:1024])
                    mv2 = ph3.tile([P, 2], F32, tag="mv2")
                    nc.vector.bn_aggr(mv2[:], stats2[:])
                    rstd2 = ph3.tile([P, 1], F32, tag="rs2")
                    nc.scalar.activation(rstd2[:], mv2[:, 1:2], AF.Sqrt,
                                         bias=eps_t[:], scale=1.0)
                    nc.vector.reciprocal(rstd2[:], rstd2[:])
                    xn2 = ph3.tile([P, C], BF16, tag="xn2")
                    nc.vector.tensor_scalar(
                        out=xn2[:], in0=x2[:, h, :], scalar1=mv2[:, 0:1],
                        scalar2=rstd2[:], op0=OP.subtract, op1=OP.mult)
                    pt2 = pp_s.tile([P, 1024], BF16, tag="ss",
                                    name=f"pt2{h}")
                    pt2v = pt2.rearrange("p (k n) -> p k n", n=P)
                    for k in range(CCH):
                        nc.tensor.transpose(pt2v[:, k, :], xn2[:, _TS(k, P)],
                                            id_b[:])
                    nc.vector.tensor_copy(ln2T[:, :, _TS(h, P)], pt2v[:])

                # software-pipelined: scores/exp of head h+1 are emitted
                # between the attention-output j-groups of head h
                ems, prev = attn_scores_emitters(0)
                for e in ems:
                    e()
                for h in range(1, NH):
                    ems, lh = attn_scores_emitters(h)
                    pso = attn_out(h - 1, prev, ems)
                    attn_post(h - 1, pso)
                    prev = lh
                pso = attn_out(NH - 1, prev)
                attn_post(NH - 1, pso)

        # ---------------- phase 4+5: MLP ------------------------------
        with ExitStack() as mctx:
            mlp = mctx.enter_context(tc.tile_pool(name="mlp", bufs=1))
            h1T = mlp.tile([P, HKN, ROWS], BF16)
            w2sb1 = mlp.tile([P, HKN, 512], BF16)
            with (
                tc.tile_pool(name="w1p", bufs=6) as w1p,
                tc.tile_pool(name="pp_m", bufs=2, space="PSUM") as pp_m,
            ):
                w1r = w1.rearrange("(k p) hh -> p k hh", p=P)
                for hk in range(HKN):
                    if hk == 6:
                        # after the first w1 tiles are queued, so MLP1
                        # starts immediately; lands well before MLP2
                        nc.sync.dma_start(w2sb1[:], w2r[:, :, 512:1024])
                    w1c = w1p.tile([P, CCH, P], BF16, tag="w1c")
                    nc.sync.dma_start(w1c[:], w1r[:, :, _TS(hk, P)])
                    psh = pp_m.tile([P, ROWS], F32, tag="mm")
                    for kc in range(CCH):
                        nc.tensor.matmul(
                            psh[:], w1c[:, kc, :], ln2T[:, kc, :],
                            start=(kc == 0), stop=(kc == CCH - 1))
                    nc.scalar.activation(
                        h1T[:, hk, :], psh[:], AF.Relu,
                        bias=b1_sb[:, hk:hk + 1], scale=1.0)

            with (
                tc.tile_pool(name="ph5", bufs=3) as ph5,
                tc.tile_pool(name="pp_f", bufs=8, space="PSUM") as pp_f,
            ):
                psf = [pp_f.tile([P, 512], F32, tag="ff", bufs=8,
                                 name=f"psf{q}") for q in range(8)]
                for q in range(8):
                    nc.tensor.matmul(
                        psf[q][:], ones_row[:], b2_sb[0:1, _TS(q // 4, 512)],
                        start=True, stop=False)
                for cg, w2half in ((0, w2sb0), (1, w2sb1)):
                    for hk in range(HKN):
                        for j in range(4):
                            nc.tensor.matmul(
                                psf[cg * 4 + j][:],
                                h1T[:, hk, _TS(j, P)],
                                w2half[:, hk, :],
                                start=False, stop=(hk == HKN - 1))
                for cg in range(2):
                    for j in range(4):
                        y_sb = ph5.tile([P, 512], F32, tag="ysb")
                        nc.vector.tensor_add(
                            y_sb[:], psf[cg * 4 + j][:],
                            x2[:, j, _TS(cg, 512)])
                        nc.sync.dma_start(y[_TS(j, P), _TS(cg, 512)],
                                          y_sb[:])


_NC_CACHE = {}


def _get_nc():
    key = os.environ.get("KERNEL_REPS", "1")
    if key not in _NC_CACHE:
        nc = bacc.Bacc("TRN2", target_bir_lowering=False, debug=False,
                       num_devices=8)
        _emit(nc)
        nc.compile()
        _NC_CACHE[key] = nc
    return _NC_CACHE[key]


def make_in_maps(x, qkv_w, qkv_b, w1, b1, w2, b2, ln1_g, ln1_b, ln2_g, ln2_b):
    import ml_dtypes
    x = np.asarray(x, dtype=np.float32)
    qkv_w = np.asarray(qkv_w, dtype=np.float32)
    qkv_b = np.asarray(qkv_b, dtype=np.float32)
    w1 = np.asarray(w1, dtype=np.float32)
    b1 = np.asarray(b1, dtype=np.float32)
    w2 = np.asarray(w2, dtype=np.float32)
    b2 = np.asarray(b2, dtype=np.float32)
    g1 = np.asarray(ln1_g, np.float32)
    bb1 = np.asarray(ln1_b, np.float32)
    g2 = np.asarray(ln2_g, np.float32)
    bb2 = np.asarray(ln2_b, np.float32)

    # Fold LN affine transforms into the downstream weights:
    #   qkv(LN1(x)) = (core1(x) * g1 + bb1) @ W + b
    #               = core1(x) @ (g1[:,None]*W) + (bb1 @ W + b)
    # and likewise LN2 into w1/b1.  The kernel then computes only the
    # core (x-mu)*rstd normalization on-chip.
    qkv_w_eff = g1[:, None] * qkv_w
    qkv_b_eff = qkv_b + bb1 @ qkv_w
    w1_eff = np.ascontiguousarray(
        (g2[:, None] * w1).astype(ml_dtypes.bfloat16))
    b1_eff = b1 + bb2 @ w1
    w2_bf = np.ascontiguousarray(w2.astype(ml_dtypes.bfloat16))

    vb_full = qkv_b_eff[2 * C:]
    in_maps = []
    for core in range(8):
        b, g = divmod(core, 4)
        cs = slice(256 * g, 256 * (g + 1))
        # Fold the v-bias into the residual input: softmax rows sum to 1,
        # so attention(v + 1*vb) = attention(v) + vb broadcast over rows.
        # In the interleaved output layout head h's vb tiles 16x along
        # the channels of its 128-row block.
        xown = x[b, 512 * g:512 * (g + 1)].copy()
        vb_core = vb_full[cs]
        for hl in range(NH):
            pat = np.tile(vb_core[64 * hl:64 * (hl + 1)], 16)
            xown[128 * hl:128 * (hl + 1), :] += pat[None, :]
        in_maps.append({
            "x": np.ascontiguousarray(x[b]),
            "xown": np.ascontiguousarray(xown),
            "wq": np.ascontiguousarray(
                qkv_w_eff[:, cs].astype(ml_dtypes.bfloat16)),
            "wk": np.ascontiguousarray(
                qkv_w_eff[:, C:2 * C][:, cs].astype(ml_dtypes.bfloat16)),
            "wv": np.ascontiguousarray(
                qkv_w_eff[:, 2 * C:][:, cs].astype(ml_dtypes.bfloat16)),
            "qb": np.ascontiguousarray(qkv_b_eff[cs]),
            "kb": np.ascontiguousarray(qkv_b_eff[C:2 * C][cs]),
            "w1": w1_eff, "b1": b1_eff, "w2": w2_bf, "b2": b2,
        })
    return in_maps


def kernel(x, qkv_w, qkv_b, w1, b1, w2, b2, ln1_g, ln1_b, ln2_g, ln2_b):
    nc = _get_nc()
    in_maps = make_in_maps(x, qkv_w, qkv_b, w1, b1, w2, b2,
                           ln1_g, ln1_b, ln2_g, ln2_b)
    res = run_bass_kernel_spmd(nc, in_maps, core_ids=list(range(8)))
    out = np.empty((B, N, C), dtype=np.float32)
    for core in range(8):
        b, g = divmod(core, 4)
        out[b, 512 * g:512 * (g + 1)] = res.results[core]["y"]
    return out
